# revision 21
# baseline (speedup 1.0000x reference)
"""Trainium2 Bass kernel: separable box filter (radius 4) on (8,3,1024,1024) fp32.

Equivalent to the reference:
    box(x) = diff(cumsum(diff(cumsum(x, H), H), W), W)    # truncated 9x9 box sum

Strategy (pure data parallel over the 24 (n,c) slices, 3 per core):
  - Overlap-free tiling: 8 blocks of 128 rows per slice, loaded exactly once
    (the previous 120-row output tiling re-read 4 halo rows per boundary,
    +6% load traffic).  Loads cast fp32 -> bf16 in the DMA (SWDGE/gpsimd
    path), which also spreads descriptors evenly over all 16 SDMA engines.
  - W pass on the DVE: one tensor_tensor_scan per block,
        state[t] = state[t-1] + xpad[t] - xpad[t-9]
    over the zero-padded row [9 zeros | x | 4 zeros], giving the truncated
    9-tap running box sum S with S[w+4] = boxW(x)[w].
  - H pass on the PE: output block b (rows 128b..128b+127) accumulates in
    PSUM up to three banded matmuls over the W-scanned blocks:
        bandB @ st_b        main 9-tap band (in-block rows)
        bandA @ st_{b-1}    rows 124..127 of the previous block -> out 0..3
        bandC @ st_{b+1}    rows 0..3 of the next block -> out 124..127
    Skipping bandA at b=0 / bandC at b=7 gives the H truncation naturally.
  - ACT copies PSUM -> SBUF (fp32); one 512KB store per block (scalar HWDGE
    ring; the b=7 stores go via the gpsimd ring so DMA engine 15 -- which
    the scalar ring never uses -- carries some of the write traffic).
"""

import numpy as np

H = 1024
W = 1024
R = 4
D = 2 * R + 1  # 9-tap window
N_CORES = 8
SLICES_PER_CORE = 3  # 8*3 = 24 (n,c) slices / 8 cores
BLK = 128
N_BLKS = 8  # 1024 / 128
P_W = D + W + R  # 9 left zeros + 1024 data + 4 right zeros
S_W = W + R  # scan output length (box sums ending at 0..1027)

_COMPILED = {}


def _band_weights():
    """Three lhsT band matrices [K=128, M=128] for the H pass:
    bandB[k, m] = 1 iff |k - m| <= 4           (in-block rows)
    bandA[k, m] = 1 iff k - 128 in [m-4, m+4]  (prev block's tail rows)
    bandC[k, m] = 1 iff k + 128 in [m-4, m+4]  (next block's head rows)
    """
    k = np.arange(BLK)[:, None]
    m = np.arange(BLK)[None, :]
    bandB = (np.abs(k - m) <= R)
    bandA = (np.abs(k - BLK - m) <= R)
    bandC = (np.abs(k + BLK - m) <= R)
    return np.stack([bandA, bandB, bandC]).astype(np.float32)


def _build():
    from concourse import bacc, mybir
    from concourse.tile import TileContext

    f32 = mybir.dt.float32
    bf16 = mybir.dt.bfloat16
    nc = bacc.Bacc("TRN2", target_bir_lowering=False, debug=False,
                   num_devices=N_CORES)

    x = nc.dram_tensor("x", (SLICES_PER_CORE, H, W), f32,
                       kind="ExternalInput").ap()
    wp = nc.dram_tensor("wp", (3, BLK, BLK), bf16,
                        kind="ExternalInput").ap()
    out = nc.dram_tensor("out", (SLICES_PER_CORE, H, W), f32,
                         kind="ExternalOutput").ap()

    add = mybir.AluOpType.add
    sub = mybir.AluOpType.subtract
    act_copy = mybir.ActivationFunctionType.Copy

    with TileContext(nc) as tc:
        with tc.tile_pool(name="wts", bufs=1) as wpool, \
             tc.tile_pool(name="xp", bufs=1) as xpool, \
             tc.tile_pool(name="sc", bufs=10) as spool, \
             tc.tile_pool(name="outp", bufs=12) as opool, \
             tc.tile_pool(name="ps", bufs=8, space="PSUM") as pspool:
            wp_t = wpool.tile([BLK, 3, BLK], bf16)
            nc.sync.dma_start(wp_t[:], wp.transpose([1, 0, 2]))
            bandA, bandB, bandC = (wp_t[:, i, :] for i in range(3))

            # 16 persistent input buffers (two full slices): slice s block b
            # uses buffer 8*(s%2)+b, so loads run a whole slice ahead of
            # compute and never stall on the previous slice's consumers.
            # Zero column pads are memset ONCE on the DVE (keeps the Pool
            # queue free to emit the first loads immediately).
            xbufs = []
            for i in range(2 * N_BLKS):
                xb = xpool.tile([BLK, P_W], bf16, tag=f"xc{i}")
                nc.vector.memset(xb[:, 0:D], 0.0)
                nc.vector.memset(xb[:, D + W:P_W], 0.0)
                xbufs.append(xb)

            for s in range(SLICES_PER_CORE):
                sts = [None] * N_BLKS

                def mm_group(b, ps_pair):
                    # Matmuls accumulating the 2D box for output rows
                    # 128b..128b+127 from the W-scanned blocks b-1, b, b+1.
                    # Skipping the bandA/bandC term at the image edges IS
                    # the H truncation.
                    for hf in range(2):
                        c0 = 512 * hf + R
                        # All matmuls use the full K=128 (the bands' zero
                        # rows drop the unwanted terms): K<=64 would select
                        # a PE row-group tile config that streams at less
                        # than half rate (537ns vs 216ns per 512 columns).
                        ops = [(bandB, sts[b][:, c0:c0 + 512])]
                        if b > 0:
                            ops.append((bandA, sts[b - 1][:, c0:c0 + 512]))
                        if b < N_BLKS - 1:
                            ops.append((bandC, sts[b + 1][:, c0:c0 + 512]))
                        ps = pspool.tile([BLK, 512], f32)
                        ps_pair.append(ps)
                        for i, (lhsT, rhs) in enumerate(ops):
                            nc.tensor.matmul(ps[:], lhsT, rhs,
                                             start=(i == 0),
                                             stop=(i == len(ops) - 1))

                def copy_store(b, ps_pair, last=False):
                    # PSUM -> SBUF, then one 512KB store per block; b=7 via
                    # the gpsimd ring so DMA engine 15 (which the scalar
                    # ring never uses) carries write traffic too.  For the
                    # kernel's final pair the DVE is already done scanning,
                    # so it takes one copy in parallel with ACT to shorten
                    # the tail.
                    oc = opool.tile([BLK, W], f32, tag="oc")
                    if last:
                        nc.vector.tensor_copy(oc[:, 0:512], ps_pair.pop(0)[:])
                        nc.scalar.activation(oc[:, 512:1024],
                                             ps_pair.pop(0)[:], act_copy)
                    else:
                        for hf in range(2):
                            nc.scalar.activation(
                                oc[:, 512 * hf:512 * hf + 512],
                                ps_pair.pop(0)[:], act_copy)
                    eng = nc.gpsimd if b == N_BLKS - 1 else nc.scalar
                    eng.dma_start(out[s, BLK * b:BLK * (b + 1), :], oc[:, :])

                def out_block_pair(b0, b1):
                    # Issue both blocks' matmuls back-to-back (10-12
                    # matmuls, dense): the PE's HAM clock gate throttles it
                    # to 1.2 GHz unless it sees ~3.4us of sustained
                    # activity, so small interleaved groups would run at
                    # half clock.
                    last = s == SLICES_PER_CORE - 1 and b1 == N_BLKS - 1
                    ps_pair = []
                    mm_group(b0, ps_pair)
                    mm_group(b1, ps_pair)
                    copy_store(b0, ps_pair, last)
                    copy_store(b1, ps_pair, last)

                final = s == SLICES_PER_CORE - 1
                ps67 = {}
                for b in range(N_BLKS):
                    xc = xbufs[N_BLKS * (s % 2) + b]
                    # fp32 DRAM -> bf16 SBUF cast during the DMA (SWDGE).
                    nc.gpsimd.dma_start(xc[:, D:D + W],
                                        x[s, BLK * b:BLK * (b + 1), :])
                    st = spool.tile([BLK, S_W], bf16)
                    nc.vector.tensor_tensor_scan(
                        st[:, :], xc[:, D:P_W], xc[:, 0:S_W], 0.0, add, sub)
                    sts[b] = st
                    if b >= 2 and b % 2 == 0:
                        out_block_pair(b - 2, b - 1)
                    if final and b == 6:
                        # Final pair, ahead-of-time half: every accumulation
                        # term that needs only st5/st6 issues now, so after
                        # the kernel's last scan only two matmuls (bandC for
                        # block 6, bandB for block 7) remain on the tail.
                        for hf in range(2):
                            c0 = 512 * hf + R
                            ps = pspool.tile([BLK, 512], f32)
                            ps67[6, hf] = ps
                            nc.tensor.matmul(ps[:], bandB,
                                             sts[6][:, c0:c0 + 512],
                                             start=True, stop=False)
                            nc.tensor.matmul(ps[:], bandA,
                                             sts[5][:, c0:c0 + 512],
                                             start=False, stop=False)
                        for hf in range(2):
                            c0 = 512 * hf + R
                            ps = pspool.tile([BLK, 512], f32)
                            ps67[7, hf] = ps
                            nc.tensor.matmul(ps[:], bandA,
                                             sts[6][:, c0:c0 + 512],
                                             start=True, stop=False)
                if final:
                    for hf in range(2):
                        c0 = 512 * hf + R
                        nc.tensor.matmul(ps67[6, hf][:], bandC,
                                         sts[7][:, c0:c0 + 512],
                                         start=False, stop=True)
                        nc.tensor.matmul(ps67[7, hf][:], bandB,
                                         sts[7][:, c0:c0 + 512],
                                         start=False, stop=True)
                    for b in (6, 7):
                        oc = opool.tile([BLK, W], f32, tag="oc")
                        nc.vector.tensor_copy(oc[:, 0:512], ps67[b, 0][:])
                        nc.scalar.activation(oc[:, 512:1024],
                                             ps67[b, 1][:], act_copy)
                        eng = nc.gpsimd if b == N_BLKS - 1 else nc.scalar
                        eng.dma_start(out[s, BLK * b:BLK * (b + 1), :],
                                      oc[:, :])
                else:
                    out_block_pair(N_BLKS - 2, N_BLKS - 1)

    nc.compile()
    return nc


def _get_nc():
    if "nc" not in _COMPILED:
        _COMPILED["nc"] = _build()
    return _COMPILED["nc"]


def _in_maps(x: np.ndarray):
    import ml_dtypes

    xf = np.ascontiguousarray(np.asarray(x, dtype=np.float32)).reshape(
        N_CORES * SLICES_PER_CORE, H, W)
    wp_np = _band_weights().astype(ml_dtypes.bfloat16)
    return [{
        "x": xf[c * SLICES_PER_CORE:(c + 1) * SLICES_PER_CORE],
        "wp": wp_np,
    } for c in range(N_CORES)]


def kernel(x: np.ndarray) -> np.ndarray:
    from concourse.bass_utils import run_bass_kernel_spmd

    nc = _get_nc()
    res = run_bass_kernel_spmd(nc, _in_maps(x), core_ids=list(range(N_CORES)))
    outs = [res.results[c]["out"] for c in range(N_CORES)]
    return np.concatenate(outs, axis=0).reshape(8, 3, H, W)


# revision 22
# speedup vs baseline: 1.1541x; 1.1541x over previous
"""Trainium2 Bass kernel: separable box filter (radius 4) on (8,3,1024,1024) fp32.

Equivalent to the reference:
    box(x) = diff(cumsum(diff(cumsum(x, H), H), W), W)    # truncated 9x9 box sum

Strategy (pure data parallel over the 24 (n,c) slices, 3 per core):
  - Overlap-free tiling: 8 blocks of 128 rows per slice, loaded exactly once
    (the previous 120-row output tiling re-read 4 halo rows per boundary,
    +6% load traffic).  Loads cast fp32 -> bf16 in the DMA (SWDGE/gpsimd
    path), which also spreads descriptors evenly over all 16 SDMA engines.
  - W pass on the DVE: one tensor_tensor_scan per block,
        state[t] = state[t-1] + xpad[t] - xpad[t-9]
    over the zero-padded row [9 zeros | x | 4 zeros], giving the truncated
    9-tap running box sum S with S[w+4] = boxW(x)[w].
  - H pass on the PE: output block b (rows 128b..128b+127) accumulates in
    PSUM up to three banded matmuls over the W-scanned blocks:
        bandB @ st_b        main 9-tap band (in-block rows)
        bandA @ st_{b-1}    rows 124..127 of the previous block -> out 0..3
        bandC @ st_{b+1}    rows 0..3 of the next block -> out 124..127
    Skipping bandA at b=0 / bandC at b=7 gives the H truncation naturally.
  - ACT copies PSUM -> SBUF (fp32); one 512KB store per block (scalar HWDGE
    ring; the b=7 stores go via the gpsimd ring so DMA engine 15 -- which
    the scalar ring never uses -- carries some of the write traffic).
"""

import numpy as np

H = 1024
W = 1024
R = 4
D = 2 * R + 1  # 9-tap window
N_CORES = 8
SLICES_PER_CORE = 3  # 8*3 = 24 (n,c) slices / 8 cores
BLK = 128
N_BLKS = 8  # 1024 / 128
P_W = D + W + R  # 9 left zeros + 1024 data + 4 right zeros
S_W = W + R  # scan output length (box sums ending at 0..1027)

_COMPILED = {}


def _band_weights():
    """Three lhsT band matrices [K=128, M=128] for the H pass:
    bandB[k, m] = 1 iff |k - m| <= 4           (in-block rows)
    bandA[k, m] = 1 iff k - 128 in [m-4, m+4]  (prev block's tail rows)
    bandC[k, m] = 1 iff k + 128 in [m-4, m+4]  (next block's head rows)
    """
    k = np.arange(BLK)[:, None]
    m = np.arange(BLK)[None, :]
    bandB = (np.abs(k - m) <= R)
    bandA = (np.abs(k - BLK - m) <= R)
    bandC = (np.abs(k + BLK - m) <= R)
    return np.stack([bandA, bandB, bandC]).astype(np.float32)


def _build():
    from concourse import bacc, mybir
    from concourse.tile import TileContext

    f32 = mybir.dt.float32
    bf16 = mybir.dt.bfloat16
    nc = bacc.Bacc("TRN2", target_bir_lowering=False, debug=False,
                   num_devices=N_CORES)

    x = nc.dram_tensor("x", (SLICES_PER_CORE, H, W), f32,
                       kind="ExternalInput").ap()
    wp = nc.dram_tensor("wp", (3, BLK, BLK), bf16,
                        kind="ExternalInput").ap()
    out = nc.dram_tensor("out", (SLICES_PER_CORE, H, W), f32,
                         kind="ExternalOutput").ap()

    add = mybir.AluOpType.add
    sub = mybir.AluOpType.subtract
    act_copy = mybir.ActivationFunctionType.Copy

    with TileContext(nc) as tc:
        with tc.tile_pool(name="wts", bufs=1) as wpool, \
             tc.tile_pool(name="xp", bufs=1) as xpool, \
             tc.tile_pool(name="sc", bufs=10) as spool, \
             tc.tile_pool(name="outp", bufs=12) as opool, \
             tc.tile_pool(name="ps", bufs=8, space="PSUM") as pspool:
            wp_t = wpool.tile([BLK, 3, BLK], bf16)
            nc.sync.dma_start(wp_t[:], wp.transpose([1, 0, 2]))
            bandA, bandB, bandC = (wp_t[:, i, :] for i in range(3))

            # 16 persistent input buffers (two full slices): slice s block b
            # uses buffer 8*(s%2)+b, so loads run a whole slice ahead of
            # compute and never stall on the previous slice's consumers.
            # Zero column pads are memset ONCE on the DVE (keeps the Pool
            # queue free to emit the first loads immediately).
            xbufs = []
            for i in range(2 * N_BLKS):
                xb = xpool.tile([BLK, P_W], bf16, tag=f"xc{i}")
                nc.vector.memset(xb[:, 0:D], 0.0)
                nc.vector.memset(xb[:, D + W:P_W], 0.0)
                xbufs.append(xb)

            for s in range(SLICES_PER_CORE):
                sts = [None] * N_BLKS

                def mm_group(b, ps_pair):
                    # Matmuls accumulating the 2D box for output rows
                    # 128b..128b+127 from the W-scanned blocks b-1, b, b+1.
                    # Skipping the bandA/bandC term at the image edges IS
                    # the H truncation.
                    for hf in range(2):
                        c0 = 512 * hf + R
                        # All matmuls use the full K=128 (the bands' zero
                        # rows drop the unwanted terms): K<=64 would select
                        # a PE row-group tile config that streams at less
                        # than half rate (537ns vs 216ns per 512 columns).
                        ops = [(bandB, sts[b][:, c0:c0 + 512])]
                        if b > 0:
                            ops.append((bandA, sts[b - 1][:, c0:c0 + 512]))
                        if b < N_BLKS - 1:
                            ops.append((bandC, sts[b + 1][:, c0:c0 + 512]))
                        ps = pspool.tile([BLK, 512], f32)
                        ps_pair.append(ps)
                        for i, (lhsT, rhs) in enumerate(ops):
                            nc.tensor.matmul(ps[:], lhsT, rhs,
                                             start=(i == 0),
                                             stop=(i == len(ops) - 1))

                def copy_store(b, ps_pair, last=False):
                    # PSUM -> SBUF, then one 512KB store per block; b=7 via
                    # the gpsimd ring so DMA engine 15 (which the scalar
                    # ring never uses) carries write traffic too.  For the
                    # kernel's final pair the DVE is already done scanning,
                    # so it takes one copy in parallel with ACT to shorten
                    # the tail.
                    oc = opool.tile([BLK, W], f32, tag="oc")
                    if last:
                        nc.vector.tensor_copy(oc[:, 0:512], ps_pair.pop(0)[:])
                        nc.scalar.activation(oc[:, 512:1024],
                                             ps_pair.pop(0)[:], act_copy)
                    else:
                        for hf in range(2):
                            nc.scalar.activation(
                                oc[:, 512 * hf:512 * hf + 512],
                                ps_pair.pop(0)[:], act_copy)
                    eng = nc.gpsimd if b == N_BLKS - 1 else nc.scalar
                    eng.dma_start(out[s, BLK * b:BLK * (b + 1), :], oc[:, :])

                def out_block_pair(b0, b1):
                    # Issue both blocks' matmuls back-to-back (10-12
                    # matmuls, dense): the PE's HAM clock gate throttles it
                    # to 1.2 GHz unless it sees ~3.4us of sustained
                    # activity, so small interleaved groups would run at
                    # half clock.
                    last = s == SLICES_PER_CORE - 1 and b1 == N_BLKS - 1
                    ps_pair = []
                    mm_group(b0, ps_pair)
                    mm_group(b1, ps_pair)
                    copy_store(b0, ps_pair, last)
                    copy_store(b1, ps_pair, last)

                for b in range(N_BLKS):
                    xc = xbufs[N_BLKS * (s % 2) + b]
                    # fp32 DRAM -> bf16 SBUF cast during the DMA (SWDGE).
                    nc.gpsimd.dma_start(xc[:, D:D + W],
                                        x[s, BLK * b:BLK * (b + 1), :])
                    st = spool.tile([BLK, S_W], bf16)
                    nc.vector.tensor_tensor_scan(
                        st[:, :], xc[:, D:P_W], xc[:, 0:S_W], 0.0, add, sub)
                    sts[b] = st
                    if b >= 2 and b % 2 == 0:
                        out_block_pair(b - 2, b - 1)
                out_block_pair(N_BLKS - 2, N_BLKS - 1)

    nc.compile()
    return nc


def _get_nc():
    if "nc" not in _COMPILED:
        _COMPILED["nc"] = _build()
    return _COMPILED["nc"]


def _in_maps(x: np.ndarray):
    import ml_dtypes

    xf = np.ascontiguousarray(np.asarray(x, dtype=np.float32)).reshape(
        N_CORES * SLICES_PER_CORE, H, W)
    wp_np = _band_weights().astype(ml_dtypes.bfloat16)
    return [{
        "x": xf[c * SLICES_PER_CORE:(c + 1) * SLICES_PER_CORE],
        "wp": wp_np,
    } for c in range(N_CORES)]


def kernel(x: np.ndarray) -> np.ndarray:
    from concourse.bass_utils import run_bass_kernel_spmd

    nc = _get_nc()
    res = run_bass_kernel_spmd(nc, _in_maps(x), core_ids=list(range(N_CORES)))
    outs = [res.results[c]["out"] for c in range(N_CORES)]
    return np.concatenate(outs, axis=0).reshape(8, 3, H, W)
